# revision 10
# baseline (speedup 1.0000x reference)
"""nn_AttentionAggregation on 8 trn2 NeuronCores (Bass/Tile), v3.

kernel(x, edge_index, att) -> [50000, 128] float32

Design (edge-parallel by dst window; no collectives):
  - fp16 node table x [50176, 128] replicated to every core as an input
    (sharding_hint allows a replicated node feature table).
  - core c owns dst rows [c*6272, (c+1)*6272) = 49 windows of 128 dst nodes.
  - phase A: per window, PE-transpose own x rows then s_dst = xT_w^T @ a_dst
    kept in SBUF (bf16).  (PE transpose; this toolchain's gpsimd ucode lacks
    the SWDGE gather/transpose library ops.)
  - phase B: per 128-edge column: one indirect DMA gathers the 128 source
    rows (256B f16 each) — the Pool-engine descriptor generation of these
    (~1.1us per column) is the hard floor of this kernel; all compute is
    batched per 7-window group / per window so it hides underneath:
      alpha_src via DVE mult+reduce over the gathered rows,
      alpha_dst via one-hot segt matmuls on PE (PSUM accumulate),
      lrelu via plain DVE ops, exp on Scalar (bf16 weights <= fp32 range),
      wx = gx * w on DVE, acc += seg^T @ [wx | w] on PE,
      normalize by the accumulated weight sums, store fp32.
  The segment-softmax max-shift cancels mathematically and fp32/bf16 range
  suffices for this distribution, so it is dropped (as in v1).
"""
import numpy as np

import concourse.bass as bass
import concourse.mybir as mybir
import concourse.tile as tile
from concourse.masks import make_identity

F32 = mybir.dt.float32
BF16 = mybir.dt.bfloat16
F16 = mybir.dt.float16
I32 = mybir.dt.int32
U8 = mybir.dt.uint8
SEG_DT = mybir.dt.bfloat16  # one-hot matrices (0/1 exact)

N_NODES = 50000
N_EDGES = 600000
C = 128
H = 8
D = 16
NCORES = 8
NS = 6272             # nodes per core (49 windows of 128)
NPAD = NS * NCORES    # 50176
NWIN = NS // 128      # 49
G = 7                 # windows per rcf-load group
NGRP = NWIN // G      # 7


# --- workaround: this container's walrus supports a single sync-wait per
# DMA/CTRL instruction; hoist extra waits onto same-engine no-ops. ---
def _split_multiwaits(nc, max_waits=1):
    for _bbname, bbwrap in nc._state.bb_map.items():
        bb = getattr(bbwrap, "bb", bbwrap)
        il = bb.instructions
        out = []
        changed = False
        for inst in il:
            si = inst.sync_info
            if si is not None and len(si.on_wait) > max_waits:
                waits = list(si.on_wait)
                keep = waits[-max_waits:]
                for w in waits[:-max_waits]:
                    nop = mybir.InstNoOp(
                        name=nc.get_next_instruction_name(), ins=[], outs=[])
                    nop.engine = inst.engine
                    nop.sync_info = mybir.SyncInfo(on_wait=[w], on_update=[])
                    nc.register_instruction(nop, overwrite=True)
                    out.append(nop)
                si.on_wait = keep
                changed = True
            out.append(inst)
        if changed:
            il[:] = out


def _host_prepare(x, edge_index, att):
    src = np.asarray(edge_index[0], dtype=np.int64)
    dst = np.asarray(edge_index[1], dtype=np.int64)

    x = np.asarray(x, dtype=np.float32)
    x_pad = np.zeros((NPAD, C), dtype=np.float32)
    x_pad[:N_NODES] = x
    xtab = x_pad.astype(np.float16)

    att = np.asarray(att, dtype=np.float32)
    w16 = np.zeros((C, 2 * H), dtype=np.float32)
    a_s, a_d = att[:, :D], att[:, D:]
    for h in range(H):
        w16[h * D:(h + 1) * D, h] = a_s[h]
        w16[h * D:(h + 1) * D, H + h] = a_d[h]
    w16 = w16.astype(np.float16)
    # a_src flattened [128] replicated per tile column for the DVE
    # mult+reduce alpha_src path
    asrc_flat = att[:, :D].reshape(1, C).astype(np.float16)

    # ---- per (core, window) edge lists ----------------------------------
    gw = dst // 128               # global window id (core*NWIN + w)
    order = np.argsort(gw, kind="stable")
    src_s = src[order]
    dst_s = dst[order]
    gw_s = gw[order]
    bounds = np.searchsorted(gw_s, np.arange(NCORES * NWIN + 1))

    cnt = (bounds[1:] - bounds[:-1]).reshape(NCORES, NWIN)
    cols = np.maximum((-(-cnt // 128)).max(axis=0), 1)  # [NWIN] max over cores

    # global column layout: window-major
    col0 = np.zeros(NWIN + 1, dtype=np.int64)
    col0[1:] = np.cumsum(cols)
    NT = int(col0[-1])
    TMAX = int(cols.max())

    meta = dict(cols=tuple(int(v) for v in cols), NT=NT, TMAX=TMAX)

    iotaP = np.arange(128, dtype=np.uint8).reshape(128, 1)
    iotaRep = np.ascontiguousarray(np.broadcast_to(
        np.tile(np.arange(128, dtype=np.uint8), TMAX), (128, TMAX * 128)))
    asrcRep = np.ascontiguousarray(np.broadcast_to(
        np.tile(asrc_flat, (1, TMAX)), (128, TMAX * C))).astype(np.float16)

    in_maps = []
    for c in range(NCORES):
        srct = np.zeros((128, NT), dtype=np.int32)
        rct = np.full((128, NT), 255, dtype=np.uint8)
        for w in range(NWIN):
            b, e = bounds[c * NWIN + w], bounds[c * NWIN + w + 1]
            n = e - b
            ncol = int(cols[w])
            ids = np.zeros(ncol * 128, dtype=np.int64)
            ids[:n] = src_s[b:e]
            r = np.full(ncol * 128, 255, dtype=np.uint8)
            r[:n] = (dst_s[b:e] % 128).astype(np.uint8)
            srct[:, col0[w]:col0[w] + ncol] = ids.reshape(ncol, 128).T
            rct[:, col0[w]:col0[w] + ncol] = r.reshape(ncol, 128).T
        rcf = np.ascontiguousarray(np.broadcast_to(
            rct.T.reshape(1, NT * 128), (128, NT * 128)))
        in_maps.append({
            "xtab": xtab,
            "xown": np.ascontiguousarray(xtab[c * NS:(c + 1) * NS]),
            "w16": w16,
            "srct": srct,
            "rct": rct,
            "rcf": rcf,
            "iotap": iotaP,
            "iotar": iotaRep,
            "asrcr": asrcRep,
        })
    return in_maps, meta


def build_kernel(meta):
    cols = meta["cols"]
    NT, TMAX = meta["NT"], meta["TMAX"]
    col0 = [0]
    for v in cols:
        col0.append(col0[-1] + v)

    nc = bass.Bass(num_devices=NCORES)

    xtab = nc.declare_dram_parameter("xtab", [NPAD, C], F16, isOutput=False)
    xown = nc.declare_dram_parameter("xown", [NS, C], F16, isOutput=False)
    w16p = nc.declare_dram_parameter("w16", [C, 2 * H], F16, isOutput=False)
    srctp = nc.declare_dram_parameter("srct", [128, NT], I32, isOutput=False)
    rctp = nc.declare_dram_parameter("rct", [128, NT], U8, isOutput=False)
    rcfp = nc.declare_dram_parameter("rcf", [128, NT * 128], U8,
                                     isOutput=False)
    iotapp = nc.declare_dram_parameter("iotap", [128, 1], U8, isOutput=False)
    iotarp = nc.declare_dram_parameter("iotar", [128, TMAX * 128], U8,
                                       isOutput=False)
    asrcrp = nc.declare_dram_parameter("asrcr", [128, TMAX * C], F16,
                                       isOutput=False)
    outp = nc.declare_dram_parameter("out", [NS, C], F32, isOutput=True)

    with tile.TileContext(nc) as tc:
        with (
            tc.tile_pool(name="const", bufs=1) as cpool,
            tc.tile_pool(name="gx", bufs=6) as gxpool,
            tc.tile_pool(name="rcfg", bufs=2) as rcfpool,
            tc.tile_pool(name="seg", bufs=3) as segpool,
            tc.tile_pool(name="segt", bufs=3) as segtpool,
            tc.tile_pool(name="prod", bufs=2) as prodpool,
            tc.tile_pool(name="wx", bufs=2) as wxpool,
            tc.tile_pool(name="sm", bufs=4) as smpool,
            tc.tile_pool(name="ot", bufs=3) as otpool,
            tc.tile_pool(name="psA", bufs=2, space="PSUM") as psApool,
            tc.tile_pool(name="psAcc", bufs=2, space="PSUM") as psAccpool,
            tc.tile_pool(name="psT", bufs=2, space="PSUM") as psTpool,
        ):
            # ---- constants ----
            ident = cpool.tile([128, 128], F16)
            make_identity(nc, ident[:])
            w16sb = cpool.tile([C, 2 * H], F16)
            nc.sync.dma_start(out=w16sb[:], in_=w16p[:])
            srct = cpool.tile([128, NT], I32)
            nc.sync.dma_start(out=srct[:], in_=srctp[:])
            rct = cpool.tile([128, NT], U8)
            nc.sync.dma_start(out=rct[:], in_=rctp[:])
            iotap = cpool.tile([128, 1], U8)
            nc.sync.dma_start(out=iotap[:], in_=iotapp[:])
            iotar = cpool.tile([128, TMAX * 128], U8)
            nc.sync.dma_start(out=iotar[:], in_=iotarp[:])
            asrcr = cpool.tile([128, TMAX * C], F16)
            nc.sync.dma_start(out=asrcr[:], in_=asrcrp[:])
            sdwt = cpool.tile([128, NWIN * H], BF16)

            # ---- phase A: per-dst-window a_dst logits ----
            with tc.tile_pool(name="pa", bufs=3) as papool:
                for w in range(NWIN):
                    xb = papool.tile([128, C], F16, tag="xb")
                    nc.sync.dma_start(
                        out=xb[:], in_=xown[w * 128:(w + 1) * 128, :])
                    xbt_ps = psTpool.tile([128, C], F16, tag="xbt")
                    nc.tensor.transpose(out=xbt_ps[:], in_=xb[:],
                                        identity=ident[:])
                    xbt = papool.tile([128, C], F16, tag="xbt_sb")
                    nc.vector.tensor_copy(out=xbt[:], in_=xbt_ps[:])
                    s_ps = psApool.tile([128, 2 * H], F32, tag="sps")
                    nc.tensor.matmul(out=s_ps[:], lhsT=xbt[:], rhs=w16sb[:],
                                     start=True, stop=True)
                    nc.vector.tensor_copy(
                        out=sdwt[:, w * H:(w + 1) * H], in_=s_ps[:, H:2 * H])

            # ---- phase B ----
            state = {}

            def emit_alpha(w, rcf_sb, gbase):
                T = cols[w]
                c0 = col0[w]
                n = T * 128
                gx = gxpool.tile([128, TMAX, C], F16, tag="gx")
                for jj in range(T):
                    nc.gpsimd.indirect_dma_start(
                        out=gx[:, jj, :], out_offset=None, in_=xtab[:],
                        in_offset=bass.IndirectOffsetOnAxis(
                            ap=srct[:, c0 + jj:c0 + jj + 1], axis=0))
                seg = segpool.tile([128, TMAX, 128], SEG_DT, tag="seg")
                nc.vector.tensor_tensor(
                    out=seg[:, 0:T, :],
                    in0=rct[:, c0:c0 + T].unsqueeze(2).to_broadcast(
                        [128, T, 128]),
                    in1=iotar[:, 0:n].rearrange("p (t e) -> p t e", e=128),
                    op=mybir.AluOpType.is_equal)
                segt = segtpool.tile([128, TMAX * 128], SEG_DT, tag="segt")
                nc.vector.tensor_tensor(
                    out=segt[:, 0:n],
                    in0=iotap[:, 0:1].to_broadcast([128, n]),
                    in1=rcf_sb[:, (c0 - gbase) * 128:(c0 - gbase) * 128 + n],
                    op=mybir.AluOpType.is_equal)
                alpha = psApool.tile([128, TMAX * H], F32, tag="alpha")
                for jj in range(T):
                    nc.tensor.matmul(
                        out=alpha[:, jj * H:(jj + 1) * H],
                        lhsT=segt[:, jj * 128:(jj + 1) * 128],
                        rhs=sdwt[:, w * H:(w + 1) * H],
                        start=True, stop=True)
                # alpha_src on DVE: prod = gx*asrc, reduce innermost 16
                prod = prodpool.tile([128, TMAX * C], F16, tag="prod")
                nc.vector.tensor_tensor(
                    out=prod[:, 0:T * C],
                    in0=gx[:, 0:T, :].rearrange("p t c -> p (t c)"),
                    in1=asrcr[:, 0:T * C],
                    op=mybir.AluOpType.mult)
                asb = smpool.tile([128, TMAX * H], F32, tag="asb")
                nc.vector.tensor_reduce(
                    out=asb[:, 0:T * H].rearrange("p (t h) -> p t h", h=H),
                    in_=prod[:, 0:T * C].rearrange(
                        "p (t h d) -> p t h d", h=H, d=D),
                    axis=mybir.AxisListType.X, op=mybir.AluOpType.add)
                state[w] = (gx, seg, alpha, asb)

            def emit_post(w):
                T = cols[w]
                T8 = T * H
                gx, seg, alpha, asb = state.pop(w)
                asum = smpool.tile([128, TMAX * H], F32, tag="asum")
                nc.vector.tensor_tensor(
                    out=asum[:, 0:T8], in0=alpha[:, 0:T8], in1=asb[:, 0:T8],
                    op=mybir.AluOpType.add)
                lm = smpool.tile([128, TMAX * H], F32, tag="lm")
                nc.vector.tensor_scalar(
                    out=lm[:, 0:T8], in0=asum[:, 0:T8], scalar1=0.2,
                    scalar2=None, op0=mybir.AluOpType.mult)
                lrl = smpool.tile([128, TMAX * H], F32, tag="lrl")
                nc.vector.tensor_tensor(
                    out=lrl[:, 0:T8], in0=asum[:, 0:T8], in1=lm[:, 0:T8],
                    op=mybir.AluOpType.max)
                wexp = smpool.tile([128, TMAX * H], BF16, tag="wexp")
                nc.scalar.activation(
                    out=wexp[:, 0:T8], in_=lrl[:, 0:T8],
                    func=mybir.ActivationFunctionType.Exp)
                wx = wxpool.tile([128, TMAX, C + H], BF16, tag="wx")
                nc.vector.tensor_copy(
                    out=wx[:, 0:T, C:C + H],
                    in_=wexp[:, 0:T8].rearrange("p (t h) -> p t h", h=H))
                nc.vector.tensor_tensor(
                    out=wx[:, 0:T, 0:C].rearrange(
                        "p t (h d) -> p t h d", h=H),
                    in0=gx[:, 0:T, :].rearrange("p t (h d) -> p t h d", h=H),
                    in1=wexp[:, 0:T8].rearrange(
                        "p (t h) -> p t h", h=H).to_broadcast(
                        [128, T, H, D]),
                    op=mybir.AluOpType.mult)
                acc = psAccpool.tile([128, C + H], F32, tag="acc")
                for jj in range(T):
                    nc.tensor.matmul(
                        out=acc[:], lhsT=seg[:, jj, :], rhs=wx[:, jj, :],
                        start=(jj == 0), stop=(jj == T - 1))
                ssum = smpool.tile([128, H], F32, tag="ssum")
                nc.vector.tensor_scalar(
                    out=ssum[:], in0=acc[:, C:C + H], scalar1=1e-10,
                    scalar2=None, op0=mybir.AluOpType.max)
                rec = smpool.tile([128, H], F32, tag="rec")
                nc.vector.reciprocal(out=rec[:], in_=ssum[:])
                ot = otpool.tile([128, C], F32, tag="ot")
                nc.vector.tensor_tensor(
                    out=ot[:].rearrange("p (h d) -> p h d", h=H),
                    in0=acc[:, 0:C].rearrange("p (h d) -> p h d", h=H),
                    in1=rec[:].to_broadcast([128, H, D]),
                    op=mybir.AluOpType.mult)
                nc.sync.dma_start(
                    out=outp[w * 128:(w + 1) * 128, :], in_=ot[:])

            prev = None
            for g in range(NGRP):
                ws = list(range(g * G, (g + 1) * G))
                gbase = col0[ws[0]]
                gc = col0[ws[-1] + 1] - gbase
                rcf_sb = rcfpool.tile([128, (TMAX * G) * 128], U8, tag="rcf")
                nc.sync.dma_start(
                    out=rcf_sb[:, 0:gc * 128],
                    in_=rcfp[:, gbase * 128:(gbase + gc) * 128])
                for w in ws:
                    emit_alpha(w, rcf_sb, gbase)
                    if prev is not None:
                        emit_post(prev)
                    prev = w
            emit_post(prev)

    _split_multiwaits(nc)
    return nc


_CACHE = {}


def kernel_with_results(x, edge_index, att, trace=False):
    import sys as _sys
    import time
    from concourse.bass_utils import run_bass_kernel_spmd
    _t = time.time()
    in_maps, meta = _host_prepare(x, edge_index, att)
    print(f"host_prepare {time.time()-_t:.1f}s NT={meta['NT']} "
          f"TMAX={meta['TMAX']}", file=_sys.stderr, flush=True)
    key = meta["cols"]
    if key not in _CACHE:
        _t = time.time()
        _CACHE[key] = build_kernel(meta)
        print(f"build_kernel {time.time()-_t:.1f}s", file=_sys.stderr,
              flush=True)
    nc = _CACHE[key]
    last = None
    for attempt in range(3):
        try:
            res = run_bass_kernel_spmd(
                nc, in_maps, list(range(NCORES)), trace=trace)
            break
        except Exception as e:  # transient device-unrecoverable under axon
            last = e
            time.sleep(20)
    else:
        raise last
    out = np.concatenate(
        [res.results[c]["out"] for c in range(NCORES)], axis=0)
    return np.ascontiguousarray(out[:N_NODES]), res


def kernel(x, edge_index, att):
    out, _ = kernel_with_results(x, edge_index, att)
    return out
